# revision 1
# baseline (speedup 1.0000x reference)
"""SNN 5-layer conv net (nn_Net_55405078118821) for 8 Trainium2 cores.

Data-parallel over batch: each core processes 4 of 32 batch elements.

Per-core dataflow (all intermediates stay in SBUF):
  - conv as banded bf16 matmuls. Spatial rows are packed 8-per-SBUF-bank
    along partitions: input bank q holds rows 8q..8q+7 as partitions
    (row%8)*Cin + cin. An output block of 8 rows needs input rows
    8q..8q+10 -> two matmuls (K from bank q, spill-K from bank q+1),
    x 4 column taps (dj, rhs free-dim offset), x n_terms precision splits
    (weights decomposed into 2-3 bf16 terms; spikes are exactly
    representable in bf16, layer-1 x is decomposed into 3 bf16 planes
    stacked along K), all accumulating into one fp32 PSUM bank of
    [M=8*Cout, 8t x Wout].
  - LIF scan per timestep over 4-bank PSUM groups:
      u_t = dv_t + u_{t-1}            (DVE tensor_tensor; u doubles as v)
      s_t = Relu(Sign(u_t - vth))     (ACT x2) -> bf16 spikes, written
                                      into the next layer's input layout
      u_t = 0 where s_t               (DVE copy_predicated)
  - layer-5 spikes DMA'd out; host computes the spatial mean.
"""

import numpy as np
import ml_dtypes

import concourse.bass as bass
import concourse.bacc as bacc
import concourse.mybir as mybir
from concourse.tile import TileContext
from concourse.bass_utils import run_bass_kernel_spmd

N_CORES = 8
B_FULL, T = 32, 16
B_LOC = B_FULL // N_CORES
F32 = mybir.dt.float32
BF16 = mybir.dt.bfloat16
NP_BF16 = ml_dtypes.bfloat16

# (Cin, Cout, Hin, Win) per layer; Hout = Hin-3, Wout = Win-3
LAYER_SHAPES = [(3, 16, 64, 64), (16, 16, 61, 61), (16, 16, 58, 58),
                (16, 16, 55, 55), (16, 6, 52, 52)]
# bf16 precision-split terms per layer (early layers amplify error most)
N_TERMS = [3, 3, 2, 2, 2]
XTERMS = 3  # layer-1 input decomposed into 3 bf16 planes stacked along K


class LayerCfg:
    def __init__(self, idx, cin, cout, hin, win):
        self.idx = idx
        self.cin, self.cout, self.hin, self.win = cin, cout, hin, win
        self.hout, self.wout = hin - 3, win - 3
        self.nbk_out = (self.hout + 7) // 8          # output banks
        self.mfull = 8 * cout                        # full-block M
        self.nsp = N_TERMS[idx]
        self.kmult = XTERMS if idx == 0 else 1       # L1: x planes along K
        self.k1max = 8 * cin * self.kmult
        self.k2max = 3 * cin * self.kmult
        self.p = min(8 * cout, 128)                  # scan partition count
        self.blocks = []
        for q in range(self.nbk_out):
            r = min(8, self.hout - 8 * q)            # rows produced
            r1 = min(8, hin - 8 * q)
            r2 = max(0, r - 5)
            self.blocks.append((q, r, r1 * cin * self.kmult,
                                r2 * cin * self.kmult))
        self.groups = [self.blocks[i:i + 4] for i in range(0, len(self.blocks), 4)]


CFGS = [LayerCfg(i, *s) for i, s in enumerate(LAYER_SHAPES)]
L5 = CFGS[-1]
SOUT_FREE = L5.nbk_out * T * L5.wout  # 7*16*49 = 5488


def _pack_A(w):
    """Banded stationary matrix for in-bank rows. w: [Cout,Cin,4,4].
    A[(rm*Cin+ci), dj*Mf + rho*Cout+co] = w[co,ci,rm-rho,dj] for 0<=rm-rho<=3."""
    cout, cin = w.shape[0], w.shape[1]
    mf = 8 * cout
    a = np.zeros((8 * cin, 4 * mf), np.float32)
    for dj in range(4):
        for rm in range(8):
            for rho in range(max(0, rm - 3), rm + 1):
                a[rm * cin:(rm + 1) * cin,
                  dj * mf + rho * cout: dj * mf + (rho + 1) * cout] = \
                    w[:, :, rm - rho, dj].T
    return a


def _pack_B(w):
    """Stationary matrix for the 3 spill rows of bank q+1 (rows 8q+8..8q+10)."""
    cout, cin = w.shape[0], w.shape[1]
    mf = 8 * cout
    b = np.zeros((3 * cin, 4 * mf), np.float32)
    for dj in range(4):
        for r8 in range(3):
            for rho in range(max(0, r8 + 5), 8):
                di = r8 + 8 - rho
                if 0 <= di <= 3:
                    b[r8 * cin:(r8 + 1) * cin,
                      dj * mf + rho * cout: dj * mf + (rho + 1) * cout] = \
                        w[:, :, di, dj].T
    return b


def _bf16_terms(a, n):
    """Decompose fp32 array into n successive bf16 remainder terms."""
    a = np.asarray(a, np.float32)
    terms = []
    for _ in range(n):
        t = a.astype(NP_BF16).astype(np.float32)
        terms.append(t)
        a = a - t
    return terms


def _expand_rows(mats):
    """Row-interleave len(mats) matrices (None -> zero rows)."""
    base = next(m for m in mats if m is not None)
    n = len(mats)
    out = np.zeros((base.shape[0] * n, base.shape[1]), np.float32)
    for j, m in enumerate(mats):
        if m is not None:
            out[j::n] = m
    return out


def _pack_layer_weights(w, cfg):
    """(wA, wB) bf16, free layout (sp, dj)-major: column offset (sp*4+dj)*Mf.
    L1: term sp applied to x planes 0..(XTERMS-1-sp) via row interleave."""
    terms = _bf16_terms(w, cfg.nsp)
    a_t = [_pack_A(t) for t in terms]
    b_t = [_pack_B(t) for t in terms]
    acols, bcols = [], []
    for sp in range(cfg.nsp):
        if cfg.idx == 0:
            pat = [a_t[sp] if sp + xj < XTERMS else None for xj in range(XTERMS)]
            acols.append(_expand_rows(pat))
            patb = [b_t[sp] if sp + xj < XTERMS else None for xj in range(XTERMS)]
            bcols.append(_expand_rows(patb))
        else:
            acols.append(a_t[sp])
            bcols.append(b_t[sp])
    wa = np.concatenate(acols, axis=1)
    wb = np.concatenate(bcols, axis=1)
    return wa.astype(NP_BF16), wb.astype(NP_BF16)


def _pack_vth_neg(vths):
    """[128, 5] per-partition NEGATED thresholds (ACT bias); p = rho*Cout+co."""
    vb = np.full((128, 5), -1e30, np.float32)
    for li, cfg in enumerate(CFGS):
        v = vths[li].reshape(-1)
        for p in range(8 * cfg.cout):
            vb[p, li] = -v[p % cfg.cout]
    return vb


_PROGRAM_CACHE = {}


def _build_program():
    if "nc" in _PROGRAM_CACHE:
        return _PROGRAM_CACHE["nc"]
    nc = bacc.Bacc("TRN2", target_bir_lowering=False, debug=False)

    # x pre-arranged on host: [b, 72 = ((h%8)*3+c)*3+plane, (h//8) x t x w] bf16
    x_d = nc.dram_tensor("xr", [B_LOC, 72, 8 * T * 64], BF16,
                         kind="ExternalInput").ap()
    wa_d, wb_d = [], []
    for li, cfg in enumerate(CFGS):
        wa_d.append(nc.dram_tensor(f"wA{li + 1}",
                                   [cfg.k1max, cfg.nsp * 4 * cfg.mfull], BF16,
                                   kind="ExternalInput").ap())
        wb_d.append(nc.dram_tensor(f"wB{li + 1}",
                                   [cfg.k2max, cfg.nsp * 4 * cfg.mfull], BF16,
                                   kind="ExternalInput").ap())
    vth_d = nc.dram_tensor("vthn", [128, 5], F32, kind="ExternalInput").ap()
    sout_d = nc.dram_tensor("sout", [B_LOC, 48, SOUT_FREE], F32,
                            kind="ExternalOutput").ap()

    with TileContext(nc) as tc:
        with (
            tc.tile_pool(name="wts", bufs=1) as wts,
            tc.tile_pool(name="xin", bufs=2) as xpool,
            tc.tile_pool(name="spk", bufs=1) as spool,
            tc.tile_pool(name="scan", bufs=3) as upool,
            tc.tile_pool(name="psum", bufs=2, space="PSUM") as ppool,
        ):
            # --- constants ---
            wa_t, wb_t = [], []
            for li, cfg in enumerate(CFGS):
                ta = wts.tile([cfg.k1max, cfg.nsp * 4 * cfg.mfull], BF16,
                              tag=f"wa{li}")
                nc.sync.dma_start(out=ta[:, :], in_=wa_d[li])
                wa_t.append(ta)
                tb = wts.tile([cfg.k2max, cfg.nsp * 4 * cfg.mfull], BF16,
                              tag=f"wb{li}")
                nc.sync.dma_start(out=tb[:, :], in_=wb_d[li])
                wb_t.append(tb)
            vth_t = wts.tile([128, 5], F32, tag="vth")
            nc.sync.dma_start(out=vth_t[:, :], in_=vth_d)
            zero_t = wts.tile([128, 244], F32, tag="zero")
            nc.any.memset(zero_t[:, :], 0.0)

            for b in range(B_LOC):
                x_t = xpool.tile([72, 8 * T * 64], BF16, tag="x")
                x_v = x_t[:, :].rearrange("p (q t w) -> p q t w", q=8, t=T)
                nc.sync.dma_start(out=x_t[:, :], in_=x_d[b])

                prev_tile, prev_view = None, None
                for li, cfg in enumerate(CFGS):
                    cout = cfg.cout
                    mf, wo = cfg.mfull, cfg.wout
                    p = cfg.p
                    s_t = spool.tile([cfg.p, cfg.nbk_out * T * wo], BF16,
                                     tag=f"s{li % 2}", name=f"s_b{b}l{li}")
                    s_v = s_t[:, :].rearrange("p (q t w) -> p q t w",
                                              q=cfg.nbk_out, t=T)
                    in_view = x_v if li == 0 else prev_view
                    nvth_ap = vth_t[0:p, li:li + 1]
                    zero3 = zero_t[:, :].rearrange(
                        "p (k w) -> p k w", w=61)
                    u_prev = [None, None]   # per-group running u (v-state)

                    for h in range(2):          # t-halves
                        for g, blocks in enumerate(cfg.groups):
                            nbk = len(blocks)
                            q0 = g * 4
                            ps = ppool.tile([128, 2048], F32, tag="ps")
                            ps_v = ps[:, :].rearrange("p (k n) -> p k n", n=512)
                            # --- conv matmuls: fill nbk banks ---
                            for bi, (q, r, k1, k2) in enumerate(blocks):
                                n = 8 * wo
                                out_ap = ps_v[0:p, bi, 0:n]
                                n_mm = cfg.nsp * 4 * (2 if k2 > 0 else 1)
                                mm = 0
                                for sp in range(cfg.nsp):
                                    for dj in range(4):
                                        c0 = (sp * 4 + dj) * mf
                                        lhs = wa_t[li][0:k1, c0:c0 + mf]
                                        rhs = in_view[0:k1, q,
                                                      h * 8:(h + 1) * 8,
                                                      dj:dj + wo]
                                        nc.tensor.matmul(
                                            out_ap, lhs, rhs,
                                            start=(mm == 0),
                                            stop=(mm == n_mm - 1))
                                        mm += 1
                                        if k2 > 0:
                                            lhs2 = wb_t[li][0:k2, c0:c0 + mf]
                                            rhs2 = in_view[0:k2, q + 1,
                                                           h * 8:(h + 1) * 8,
                                                           dj:dj + wo]
                                            nc.tensor.matmul(
                                                out_ap, lhs2, rhs2,
                                                start=False,
                                                stop=(mm == n_mm - 1))
                                            mm += 1
                            # --- LIF scan over this half's 8 timesteps ---
                            for t in range(8):
                                tt = h * 8 + t
                                dv = ps_v[0:p, 0:nbk, t * wo:(t + 1) * wo]
                                u_t = upool.tile([cfg.p, 4 * wo], F32,
                                                 tag=f"u{g}",
                                                 name=f"u_b{b}l{li}g{g}t{tt}")
                                u_v = u_t[:, :].rearrange(
                                    "p (k w) -> p k w", w=wo)[0:p, 0:nbk, :]
                                v_in = zero3[0:p, 0:nbk, 0:wo] if tt == 0 \
                                    else u_prev[g]
                                nc.vector.tensor_tensor(
                                    out=u_v, in0=dv, in1=v_in,
                                    op=mybir.AluOpType.add)
                                sg_t = upool.tile([cfg.p, 4 * wo], BF16,
                                                  tag="sg",
                                                  name=f"sg_b{b}l{li}g{g}t{tt}")
                                sg_v = sg_t[:, :].rearrange(
                                    "p (k w) -> p k w", w=wo)[0:p, 0:nbk, :]
                                nc.scalar.activation(
                                    sg_v, u_v,
                                    mybir.ActivationFunctionType.Sign,
                                    bias=nvth_ap)
                                s_out = s_v[0:p, q0:q0 + nbk, tt, :]
                                nc.scalar.activation(
                                    s_out, sg_v,
                                    mybir.ActivationFunctionType.Relu)
                                z_t = upool.tile([cfg.p, 4 * wo], BF16,
                                                 tag="zz",
                                                 name=f"z_b{b}l{li}g{g}t{tt}")
                                z_v = z_t[:, :].rearrange(
                                    "p (k w) -> p k w", w=wo)[0:p, 0:nbk, :]
                                nc.scalar.activation(
                                    z_v, sg_v,
                                    mybir.ActivationFunctionType.Relu,
                                    scale=-1.0)
                                nc.vector.tensor_tensor(
                                    out=u_v, in0=u_v, in1=z_v,
                                    op=mybir.AluOpType.mult)
                                u_prev[g] = u_v
                    prev_tile, prev_view = s_t, s_v

                # --- emit layer-5 spikes (bf16 -> f32 DRAM via casting DMA) ---
                nc.gpsimd.dma_start(out=sout_d[b], in_=prev_tile[0:48, :])

    nc.compile()
    _PROGRAM_CACHE["nc"] = nc
    return nc


def _arrange_x(x):
    """[b,T,3,64,64] -> bf16 [b, 72 = ((h%8)*3+c)*3+plane, (h//8) x t x w]."""
    bl = x.shape[0]
    x = np.ascontiguousarray(x, np.float32)
    planes = _bf16_terms(x, XTERMS)
    xs = np.stack(planes, axis=-1)              # b t c h w sp
    xs = xs.reshape(bl, T, 3, 8, 8, 64, XTERMS)  # b t c q hm w sp
    xs = xs.transpose(0, 4, 2, 6, 3, 1, 5)      # b hm c sp q t w
    return np.ascontiguousarray(
        xs.reshape(bl, 72, 8 * T * 64).astype(NP_BF16))


def _host_inputs(inputs):
    m = {}
    for li, cfg in enumerate(CFGS):
        wa, wb = _pack_layer_weights(np.asarray(inputs[f"w{li + 1}"], np.float32),
                                     cfg)
        m[f"wA{li + 1}"] = wa
        m[f"wB{li + 1}"] = wb
    m["vthn"] = _pack_vth_neg(
        [np.asarray(inputs[f"vth{i + 1}"], np.float32) for i in range(5)])
    return m


def decode_sout(sout):
    """[B_LOC, 48, SOUT_FREE] -> [B_LOC, T, 6] spike means."""
    a = sout.reshape(B_LOC, 8, 6, L5.nbk_out, T, L5.wout)
    rho = np.arange(8)[:, None]
    qq = np.arange(L5.nbk_out)[None, :]
    mask = (8 * qq + rho) < L5.hout                     # [rho, q]
    a = a.transpose(0, 4, 2, 1, 3, 5)                   # [b, t, c, rho, q, j]
    vals = a[:, :, :, mask, :]                          # [b, t, c, 49, 49]
    return vals.mean(axis=(3, 4)).astype(np.float32)


def run_spmd(inputs, **kw):
    nc = _build_program()
    x = np.asarray(inputs["x"], np.float32)
    const = _host_inputs(inputs)
    in_maps = []
    for c in range(N_CORES):
        m = dict(const)
        m["xr"] = _arrange_x(x[c * B_LOC:(c + 1) * B_LOC])
        in_maps.append(m)
    return run_bass_kernel_spmd(nc, in_maps, list(range(N_CORES)), **kw)


def kernel(**inputs):
    res = run_spmd(inputs)
    outs = [decode_sout(r["sout"]) for r in res.results]
    return np.concatenate(outs, axis=0)

